# revision 1
# baseline (speedup 1.0000x reference)
"""RQSplineHead Trainium2 Bass kernel.

Computes log-prob histogram of a rational-quadratic-spline CDF head:
  params = softplus(h @ W.T + b) + 1e-4           [B, 27]
  w, hh, d = split(params); w,hh normalized; cumx/cumy = padded cumsums
  F = RQS CDF evaluated at 129 uniform edges; out = log(clip(diff(F), 1e-8))

Strategy (pure data parallel over 8 cores, 16384 rows/core):
  - rows live on SBUF partitions (128-row blocks)
  - PE transposes h-tiles and does the 256x27 matmul (+bias via ones-row)
  - ACT does softplus as Ln(Exp(z) + 1) (both funcs in one table set)
  - All spline math is done in the unnormalized variable Xt = x * Sw
    (Sw = sum of unnormalized widths), so no per-element normalization
    is needed.  Per bin j the CDF is the ratio of quadratics in the local
    coordinate u = Xt - a_bin:
        G = Sh*F = (P0 + P1*u + P2*u^2) / (C0 + C1*u + C2*u^2)
    The 7 piecewise-constant-per-bin quantities (a, P0..P2, C0..C2) are
    built per (row, edge) with exact step-mask accumulation chains:
        q(r,e) = q_bin0(r) + sum_j [Xt(r,e) > cx_j(r)] * (q_j - q_{j-1})(r)
    each term is ONE fused tensor_scalar (is_gt, mult) with per-partition
    (=per-row) scalar operands.  Chains are split between DVE and GPSIMD.
  - log(probs) = Ln(clip(diff(G), 1e-8*Sh) * (1/Sh)) on ACT with a
    per-partition reciprocal scale.
"""

import numpy as np
from contextlib import ExitStack

import concourse.bass as bass
import concourse.mybir as mybir
import concourse.tile as tile
from concourse.masks import make_identity

f32 = mybir.dt.float32
i32 = mybir.dt.int32
OP = mybir.AluOpType
AF = mybir.ActivationFunctionType

B_FULL = 131072
IN_DIM = 256
NE = 129          # edges (n_bins + 1)
NB = 128          # output bins
NK = 9            # spline bins per row (n_knots + 1)
ODIM = 27
N_CORES = 8
P = 128


def build_rqs(ctx: ExitStack, tc: "tile.TileContext", h, W, b, out, rows,
              gpsimd_chains=(4, 5, 6)):
    """Emit the kernel for `rows` rows (one core's shard).

    gpsimd_chains: indices (0..6) of chains run on GPSIMD instead of DVE.
      chain order: 0:u(a) 1:P0 2:P1 3:P2 4:C0 5:C1 6:C2
    """
    nc = tc.nc
    nblk = rows // P
    G = min(8, nblk)
    nsb = nblk // G
    assert nsb * G == nblk, (rows, nblk, G)

    const = ctx.enter_context(tc.tile_pool(name="const", bufs=1))
    psum = ctx.enter_context(tc.tile_pool(name="psum", bufs=2, space="PSUM"))
    psum1 = ctx.enter_context(tc.tile_pool(name="psum1", bufs=1, space="PSUM"))
    hpool = ctx.enter_context(tc.tile_pool(name="hpool", bufs=3))
    bs = ctx.enter_context(tc.tile_pool(name="bs", bufs=2))
    es = ctx.enter_context(tc.tile_pool(name="es", bufs=2))
    outp = ctx.enter_context(tc.tile_pool(name="outp", bufs=3))

    # ---------------- constants ----------------
    ident = const.tile([P, P], f32)
    make_identity(nc, ident)

    # x grid as f32: xconst[p, e] = e / 128
    xi = const.tile([P, NE], i32)
    nc.gpsimd.iota(xi, pattern=[[1, NE]], base=0, channel_multiplier=0)
    xconst = const.tile([P, NE], f32)
    nc.vector.tensor_scalar(xconst, xi, 1.0 / NB, None, op0=OP.mult)

    # scan gate: 1.0 everywhere except 0.0 at each 9-wide section start
    gate = const.tile([P, G, NK], f32)
    nc.vector.memset(gate, 1.0)
    nc.vector.memset(gate[:, :, 0:1], 0.0)

    ones1 = const.tile([1, P], f32)
    nc.vector.memset(ones1, 1.0)

    # W^T chunks [128 feat, 27] x2 and bias row [1, 27]
    wraw = const.tile([P, IN_DIM], f32)
    nc.vector.memset(wraw, 0.0)
    nc.sync.dma_start(out=wraw[0:ODIM, :], in_=W)
    psw = psum1.tile([P, 2, P], f32)
    for k in range(2):
        nc.tensor.transpose(psw[:, k], wraw[:, k * P:(k + 1) * P], ident)
    wT = const.tile([P, 2, ODIM], f32)
    nc.scalar.copy(wT, psw[:, :, 0:ODIM])
    brow = const.tile([1, ODIM], f32)
    nc.sync.dma_start(out=brow, in_=b.rearrange("(o k) -> o k", o=1))

    for sb in range(nsb):
        # ---------------- phase 1: params = softplus(h @ W.T + b) ----------
        params = bs.tile([P, G, ODIM], f32, tag="params")
        usb = bs.tile([P, G, ODIM], f32, tag="usb")
        for g in range(G):
            blk = sb * G + g
            r0 = blk * P
            ht = hpool.tile([P, IN_DIM], f32, tag="ht")
            nc.sync.dma_start(out=ht, in_=h[r0:r0 + P, :])
            psT = psum.tile([P, 2, P], f32, tag="psT")
            for k in range(2):
                nc.tensor.transpose(psT[:, k], ht[:, k * P:(k + 1) * P], ident)
            hT = hpool.tile([P, 2, P], f32, tag="hT")
            nc.scalar.copy(hT, psT)
            pp = psum.tile([P, ODIM], f32, tag="pp")
            nc.tensor.matmul(pp, hT[:, 0], wT[:, 0], start=True, stop=False)
            nc.tensor.matmul(pp, hT[:, 1], wT[:, 1], start=False, stop=False)
            nc.tensor.matmul(pp, ones1, brow, start=False, stop=True)
            expt = usb[:, g]
            nc.scalar.activation(expt, pp, AF.Exp)
            # the HW Exp table is only ~1e-5 accurate; one Newton step off the
            # (much more accurate) Ln table: u1 = u0*(1 + (z - ln(u0)))
            lu = hpool.tile([P, ODIM], f32, tag="lu")
            nc.scalar.activation(lu, expt, AF.Ln)
            dz = hpool.tile([P, ODIM], f32, tag="dz")
            nc.vector.tensor_tensor(dz, pp, lu, OP.subtract)
            nc.vector.tensor_tensor(dz, expt, dz, OP.mult)
            nc.vector.tensor_tensor(expt, expt, dz, OP.add)
            # softplus: Ln(exp(z) + 1)
            nc.scalar.activation(params[:, g], expt, AF.Ln, bias=1.0, scale=1.0)

        # Ln(1+u) via table floors at ~ulp(1) absolute, which is a large
        # relative error when softplus(z) ~ 1e-6 (it then competes with the
        # +1e-4 epsilon).  For u < 1/16 use the log1p series directly.
        u2sb = bs.tile([P, G, ODIM], f32, tag="u2sb")
        nc.vector.tensor_tensor(u2sb, usb, usb, OP.mult)
        pA = bs.tile([P, G, ODIM], f32, tag="pA")
        nc.vector.tensor_scalar(pA, usb, -0.5, 1.0, op0=OP.mult, op1=OP.add)
        pB = bs.tile([P, G, ODIM], f32, tag="pB")
        nc.vector.tensor_scalar(pB, usb, -0.25, 1.0 / 3, op0=OP.mult, op1=OP.add)
        pC = bs.tile([P, G, ODIM], f32, tag="pC")
        nc.vector.tensor_scalar(pC, usb, -1.0 / 6, 0.2, op0=OP.mult, op1=OP.add)
        u4sb = bs.tile([P, G, ODIM], f32, tag="u4sb")
        nc.vector.tensor_tensor(u4sb, u2sb, u2sb, OP.mult)
        nc.vector.tensor_tensor(pB, pB, u2sb, OP.mult)
        nc.vector.tensor_tensor(pA, pA, pB, OP.add)
        nc.vector.tensor_tensor(pC, pC, u4sb, OP.mult)
        nc.vector.tensor_tensor(pA, pA, pC, OP.add)
        nc.vector.tensor_tensor(pA, pA, usb, OP.mult)
        msk = bs.tile([P, G, ODIM], mybir.dt.uint8, tag="msk")
        nc.vector.tensor_scalar(msk, usb, 0.0625, None, op0=OP.is_lt)
        nc.vector.copy_predicated(params, msk, pA)

        # ---------------- phase 2: bin-space coefficients -------------------
        w_in = bs.tile([P, G, NK], f32, tag="w_in")
        nc.vector.tensor_scalar(w_in, params[:, :, 0:NK], 1e-4, None, op0=OP.add)
        h_in = bs.tile([P, G, NK], f32, tag="h_in")
        nc.vector.tensor_scalar(h_in, params[:, :, NK:2 * NK], 1e-4, None, op0=OP.add)
        dpad = bs.tile([P, G, NK + 2], f32, tag="dpad")
        nc.vector.memset(dpad, 1.0)
        nc.vector.tensor_scalar(
            dpad[:, :, 1:NK + 1], params[:, :, 2 * NK:3 * NK], 1e-4, None, op0=OP.add)

        cx = bs.tile([P, G, NK], f32, tag="cx")
        nc.vector.tensor_tensor_scan(
            cx.rearrange("p g k -> p (g k)"),
            gate.rearrange("p g k -> p (g k)"),
            w_in.rearrange("p g k -> p (g k)"),
            0.0, op0=OP.mult, op1=OP.add)
        cy = bs.tile([P, G, NK], f32, tag="cy")
        nc.vector.tensor_tensor_scan(
            cy.rearrange("p g k -> p (g k)"),
            gate.rearrange("p g k -> p (g k)"),
            h_in.rearrange("p g k -> p (g k)"),
            0.0, op0=OP.mult, op1=OP.add)

        rw = bs.tile([P, G, NK], f32, tag="rw")
        nc.vector.reciprocal(rw, w_in)
        rsh = bs.tile([P, G], f32, tag="rsh")
        nc.vector.reciprocal(rsh, cy[:, :, NK - 1])
        lam = bs.tile([P, G], f32, tag="lam")
        nc.vector.tensor_tensor(lam, cx[:, :, NK - 1], rsh, OP.mult)

        # delta_hat = lam * h/w  (lam multiply done per-block: per-partition scalar)
        dd = bs.tile([P, G, NK], f32, tag="dd")
        nc.vector.tensor_tensor(dd, h_in, rw, OP.mult)
        for g in range(G):
            nc.vector.tensor_scalar(dd[:, g], dd[:, g], lam[:, g:g + 1], None,
                                    op0=OP.mult)

        dl = dpad[:, :, 0:NK]
        dr = dpad[:, :, 1:NK + 1]
        s_t = bs.tile([P, G, NK], f32, tag="s_t")
        nc.vector.tensor_tensor(s_t, dl, dr, OP.add)
        nc.vector.scalar_tensor_tensor(s_t, dd, -2.0, s_t, op0=OP.mult, op1=OP.add)

        apc = bs.tile([P, G, NK], f32, tag="apc")     # A' = h*(dd - dl)
        nc.vector.tensor_tensor(apc, dd, dl, OP.subtract)
        nc.vector.tensor_tensor(apc, apc, h_in, OP.mult)
        bpc = bs.tile([P, G, NK], f32, tag="bpc")     # B' = h*dl*w
        nc.vector.tensor_tensor(bpc, h_in, dl, OP.mult)
        nc.vector.tensor_tensor(bpc, bpc, w_in, OP.mult)

        w2 = bs.tile([P, G, NK], f32, tag="w2")
        nc.vector.tensor_tensor(w2, w_in, w_in, OP.mult)
        c0 = bs.tile([P, G, NK], f32, tag="c0")
        nc.vector.tensor_tensor(c0, dd, w2, OP.mult)
        c1 = bs.tile([P, G, NK], f32, tag="c1")
        nc.vector.tensor_tensor(c1, s_t, w_in, OP.mult)
        c2 = bs.tile([P, G, NK], f32, tag="c2")
        nc.vector.tensor_scalar(c2, s_t, -1.0, None, op0=OP.mult)

        # chain order: 0:a (jumps w) 1:cy (jumps h) 2:A' 3:B' 4:C0 5:C1 6:C2
        coefs = [None, None, apc, bpc, c0, c1, c2]
        dts = [w_in, h_in]
        for ci in range(2, 7):
            q = coefs[ci]
            dq = bs.tile([P, G, NK - 1], f32, tag=f"dq{ci}")
            nc.vector.tensor_tensor(dq, q[:, :, 1:NK], q[:, :, 0:NK - 1],
                                    OP.subtract)
            dts.append(dq)

        thr = bs.tile([P, G], f32, tag="thr")
        nc.vector.tensor_scalar(thr, cy[:, :, NK - 1], 1e-8, None, op0=OP.mult)

        # ---------------- phase 3: edge-space evaluation ---------------------
        for g in range(G):
            blk = sb * G + g
            r0 = blk * P
            sw = cx[:, g, NK - 1:NK]

            xt = es.tile([P, NE], f32, tag="xt")
            nc.vector.tensor_scalar(xt, xconst, sw, None, op0=OP.mult)

            def chain_scalar(ci, j):
                # scalar operands for chain ci, step j (j = 1..8)
                return cx[:, g, j - 1:j], dts[ci][:, g, j - 1:j]

            # mask-madd terms always on DVE (Pool lacks TensorScalarPtr);
            # accumulation adds of gpsimd_chains run on GPSIMD.
            accs = []
            for ci in range(7):
                on_gp = ci in gpsimd_chains
                adder = nc.gpsimd if on_gp else nc.vector
                acc = es.tile([P, NE], f32, tag=f"acc{ci}")
                if on_gp:
                    terms = [es.tile([P, NE], f32, tag=f"trm{ci}_{j}",
                                     name=f"trm{ci}_{j}")
                             for j in range(1, NK)]
                    for j in range(1, NK):
                        cxs, dq = chain_scalar(ci, j)
                        nc.vector.tensor_scalar(terms[j - 1], xt, cxs, dq,
                                                op0=OP.is_gt, op1=OP.mult)
                    adder.tensor_tensor(acc, terms[0], terms[1], OP.add)
                    for j in range(2, NK - 1):
                        adder.tensor_tensor(acc, acc, terms[j], OP.add)
                else:
                    tmp = es.tile([P, NE], f32, tag=f"tmpc{ci}")
                    for j in range(1, NK):
                        cxs, dq = chain_scalar(ci, j)
                        dst = acc if j == 1 else tmp
                        nc.vector.tensor_scalar(dst, xt, cxs, dq,
                                                op0=OP.is_gt, op1=OP.mult)
                        if j > 1:
                            nc.vector.tensor_tensor(acc, acc, tmp, OP.add)
                accs.append(acc)

            # u = xt - a_chain
            u = es.tile([P, NE], f32, tag="u")
            nc.vector.tensor_tensor(u, xt, accs[0], OP.subtract)
            u2 = es.tile([P, NE], f32, tag="u2")
            nc.vector.tensor_tensor(u2, u, u, OP.mult)

            # numerator of the rational increment: u*((A'b+A'ch)*u + B'b+B'ch)
            m = es.tile([P, NE], f32, tag="m")
            nc.vector.scalar_tensor_tensor(
                m, accs[2], apc[:, g, 0:1], u, op0=OP.add, op1=OP.mult)
            nc.vector.scalar_tensor_tensor(
                m, accs[3], bpc[:, g, 0:1], m, op0=OP.add, op1=OP.add)
            nc.vector.tensor_tensor(m, m, u, OP.mult)

            # denominator D = C0 + C1*u + C2*u^2 with per-bin bases + chains
            dacc = es.tile([P, NE], f32, tag="dacc")
            nc.vector.tensor_scalar(dacc, u, c1[:, g, 0:1], c0[:, g, 0:1],
                                    op0=OP.mult, op1=OP.add)
            nc.vector.scalar_tensor_tensor(dacc, u2, c2[:, g, 0:1], dacc,
                                           op0=OP.mult, op1=OP.add)
            nc.gpsimd.tensor_tensor(dacc, dacc, accs[4], OP.add)
            t2 = es.tile([P, NE], f32, tag="t2")
            nc.gpsimd.tensor_tensor(t2, u, accs[5], OP.mult)
            nc.gpsimd.tensor_tensor(dacc, dacc, t2, OP.add)
            t3 = es.tile([P, NE], f32, tag="t3")
            nc.gpsimd.tensor_tensor(t3, u2, accs[6], OP.mult)
            nc.gpsimd.tensor_tensor(dacc, dacc, t3, OP.add)

            rd = es.tile([P, NE], f32, tag="rd")
            scr = es.tile([P, NE], f32, tag="scr")
            nc.vector.reciprocal_approx_accurate(rd, dacc, scr)
            rat = es.tile([P, NE], f32, tag="rat")
            nc.vector.tensor_tensor(rat, m, rd, OP.mult)

            # probs~ = diff(cy_chain) + diff(rat); within-bin cy diffs cancel
            # bit-exactly, keeping the difference at increment scale.
            pt = outp.tile([P, NB], f32, tag="pt")
            nc.vector.tensor_tensor(pt, accs[1][:, 1:NE], accs[1][:, 0:NB],
                                    OP.subtract)
            dr = outp.tile([P, NB], f32, tag="dr")
            nc.vector.tensor_tensor(dr, rat[:, 1:NE], rat[:, 0:NB],
                                    OP.subtract)
            nc.vector.tensor_tensor(pt, pt, dr, OP.add)
            nc.vector.tensor_scalar(pt, pt, thr[:, g:g + 1], None, op0=OP.max)
            ot = outp.tile([P, NB], f32, tag="ot")
            nc.scalar.activation(ot, pt, AF.Ln, bias=0.0, scale=rsh[:, g:g + 1])
            nc.sync.dma_start(out=out[r0:r0 + P, :], in_=ot)


def make_nc(rows, gpsimd_chains=(4, 5, 6)):
    import concourse.bacc as bacc
    nc = bacc.Bacc("TRN2", target_bir_lowering=False, debug=False,
                   num_devices=N_CORES)
    h_t = nc.dram_tensor("h", [rows, IN_DIM], f32, kind="ExternalInput").ap()
    W_t = nc.dram_tensor("W", [ODIM, IN_DIM], f32, kind="ExternalInput").ap()
    b_t = nc.dram_tensor("b", [ODIM], f32, kind="ExternalInput").ap()
    out_t = nc.dram_tensor("out", [rows, NB], f32, kind="ExternalOutput").ap()
    with tile.TileContext(nc) as tc:
        with ExitStack() as ctx:
            build_rqs(ctx, tc, h_t, W_t, b_t, out_t, rows,
                      gpsimd_chains=gpsimd_chains)
    nc.compile()
    return nc


_cache = {}


def kernel(h, W, b):
    h = np.ascontiguousarray(h, dtype=np.float32)
    W = np.ascontiguousarray(W, dtype=np.float32)
    b = np.ascontiguousarray(b, dtype=np.float32)
    rows = h.shape[0] // N_CORES
    key = ("nc", rows)
    if key not in _cache:
        _cache[key] = make_nc(rows)
    nc = _cache[key]
    from concourse.bass_utils import run_bass_kernel_spmd
    in_maps = [
        {"h": h[i * rows:(i + 1) * rows], "W": W, "b": b}
        for i in range(N_CORES)
    ]
    res = run_bass_kernel_spmd(nc, in_maps, core_ids=list(range(N_CORES)))
    return np.concatenate([r["out"] for r in res.results], axis=0)


if __name__ == "__main__":
    rng = np.random.default_rng(0)
    h = rng.standard_normal((B_FULL, IN_DIM), dtype=np.float32)
    W = (rng.standard_normal((ODIM, IN_DIM), dtype=np.float32) / 16.0)
    b = rng.standard_normal((ODIM,), dtype=np.float32) * 0.01
    out = kernel(h, W, b)
    print(out.shape, out.dtype, out[:2, :4])



# revision 3
# speedup vs baseline: 1.1052x; 1.1052x over previous
"""RQSplineHead Trainium2 Bass kernel.

Computes log-prob histogram of a rational-quadratic-spline CDF head:
  params = softplus(h @ W.T + b) + 1e-4           [B, 27]
  w, hh, d = split(params); w,hh normalized; cumx/cumy = padded cumsums
  F = RQS CDF evaluated at 129 uniform edges; out = log(clip(diff(F), 1e-8))

Strategy (pure data parallel over 8 cores, 16384 rows/core):
  - rows live on SBUF partitions (128-row blocks)
  - PE transposes h-tiles and does the 256x27 matmul (+bias via ones-row)
  - ACT does softplus as Ln(Exp(z) + 1) (both funcs in one table set)
  - All spline math is done in the unnormalized variable Xt = x * Sw
    (Sw = sum of unnormalized widths).  Per bin j the CDF is the ratio of
    quadratics in the local coordinate u = Xt - a_bin:
        G = Sh*F = cy + u*(A'*u + B') / (C0 + C1*u + C2*u^2)
    The 7 piecewise-constant-per-bin quantities (a, cy, A', B', C0..C2)
    are built per (row, edge) with step-mask accumulation chains:
        q(r,e) = q_bin0(r) + sum_j [Xt(r,e) > cx_j(r)] * (q_j - q_{j-1})(r)
    Each step is ONE custom fused DVE op (STEP_MADD_ANT:
    out = in1 + (in0 > s0)*s1, per-partition scalar operands), so a chain
    is 8 DVE instructions with no separate mask or add passes.
  - The rational evaluation (u, numerator, denominator, Newton-refined
    reciprocal, diffs) runs on GPSIMD tensor_tensor ops; Xt, u^2 and the
    final Ln run on ACT (Copy/Square/Ln share one activation table set),
    keeping DVE free for the chains.
  - log(probs) = Ln(clip(diff(G), 1e-8*Sh) * (1/Sh)) on ACT with a
    per-partition reciprocal scale.
"""

import numpy as np
from contextlib import ExitStack

import concourse.bass as bass
import concourse.mybir as mybir
import concourse.tile as tile
from concourse.masks import make_identity

f32 = mybir.dt.float32
i32 = mybir.dt.int32
OP = mybir.AluOpType
AF = mybir.ActivationFunctionType

B_FULL = 131072
IN_DIM = 256
NE = 129          # edges (n_bins + 1)
NB = 128          # output bins
NK = 9            # spline bins per row (n_knots + 1)
ODIM = 27
N_CORES = 8
P = 128


# ---------------------------------------------------------------------------
# Custom fused DVE op: out = in1 + (in0 > s0) * s1   (s0, s1 per-partition)
# One chain step in a single DVE instruction.
# ---------------------------------------------------------------------------
def _make_step_op():
    from concourse import dve_ops as DO
    from concourse import dve_spec as DS
    from concourse.dve_uop import DveOpSpec

    name = "STEP_MADD_ANT"
    for op in DO.OPS:
        if op.name == name:
            return op
    spec = DS.Spec(
        body=DS.Src1 + (DS.Src0 > DS.C0) * DS.C1,
        reference=lambda in0, in1, s0, s1, imm2: in1 + (in0 > s0) * s1,
    )
    row = max(DO._SUB_OPCODE_FOR_NAME.values()) + 1
    assert row < 0x20, "no free custom-DVE opcode rows"
    DO._SUB_OPCODE_FOR_NAME[name] = row
    shas = {}
    for ver in ("v3", "v4"):
        try:
            s = DveOpSpec(name=name, opcode=row,
                          uops=DS.lower(spec, ver=ver), rd1_en=True)
            shas[ver] = s.sha(ver)
        except Exception:
            pass
    assert shas, "failed to lower STEP_MADD_ANT for any DVE version"
    op = DO.DveOp(name, spec, subdim=False, uops_sha=shas)
    DO.OPS.append(op)
    DO.CUSTOM_DVE_SPECS[name] = spec
    return op


STEP_MADD = _make_step_op()


def build_rqs(ctx: ExitStack, tc: "tile.TileContext", h, W, b, out, rows):
    """Emit the kernel for `rows` rows (one core's shard)."""
    nc = tc.nc
    nblk = rows // P
    G = min(8, nblk)
    nsb = nblk // G
    assert nsb * G == nblk, (rows, nblk, G)

    def step_madd(acc, xt, cxs, dq):
        nc.vector._custom_dve(STEP_MADD, out=acc, in0=xt, in1=acc,
                              s0=cxs, s1=dq)

    const = ctx.enter_context(tc.tile_pool(name="const", bufs=1))
    psum = ctx.enter_context(tc.tile_pool(name="psum", bufs=2, space="PSUM"))
    psum1 = ctx.enter_context(tc.tile_pool(name="psum1", bufs=1, space="PSUM"))
    hpool = ctx.enter_context(tc.tile_pool(name="hpool", bufs=3))
    bs = ctx.enter_context(tc.tile_pool(name="bs", bufs=2))
    es = ctx.enter_context(tc.tile_pool(name="es", bufs=2))
    outp = ctx.enter_context(tc.tile_pool(name="outp", bufs=3))

    # ---------------- constants ----------------
    ident = const.tile([P, P], f32)
    make_identity(nc, ident)

    # x grid as f32: xconst[p, e] = e / 128
    xi = const.tile([P, NE], i32)
    nc.gpsimd.iota(xi, pattern=[[1, NE]], base=0, channel_multiplier=0)
    xconst = const.tile([P, NE], f32)
    nc.vector.tensor_scalar(xconst, xi, 1.0 / NB, None, op0=OP.mult)

    # scan gate: 1.0 everywhere except 0.0 at each 9-wide section start
    gate = const.tile([P, G, NK], f32)
    nc.vector.memset(gate, 1.0)
    nc.vector.memset(gate[:, :, 0:1], 0.0)

    ones1 = const.tile([1, P], f32)
    nc.vector.memset(ones1, 1.0)

    two_const = const.tile([P, NE], f32)
    nc.vector.memset(two_const, 2.0)

    # W^T chunks [128 feat, 27] x2 and bias row [1, 27]
    wraw = const.tile([P, IN_DIM], f32)
    nc.vector.memset(wraw, 0.0)
    nc.sync.dma_start(out=wraw[0:ODIM, :], in_=W)
    psw = psum1.tile([P, 2, P], f32)
    for k in range(2):
        nc.tensor.transpose(psw[:, k], wraw[:, k * P:(k + 1) * P], ident)
    wT = const.tile([P, 2, ODIM], f32)
    nc.scalar.copy(wT, psw[:, :, 0:ODIM])
    brow = const.tile([1, ODIM], f32)
    nc.sync.dma_start(out=brow, in_=b.rearrange("(o k) -> o k", o=1))

    for sb in range(nsb):
        # ---------------- phase 1: params = softplus(h @ W.T + b) ----------
        params = bs.tile([P, G, ODIM], f32, tag="params")
        usb = bs.tile([P, G, ODIM], f32, tag="usb")
        for g in range(G):
            blk = sb * G + g
            r0 = blk * P
            ht = hpool.tile([P, IN_DIM], f32, tag="ht")
            nc.sync.dma_start(out=ht, in_=h[r0:r0 + P, :])
            psT = psum.tile([P, 2, P], f32, tag="psT")
            for k in range(2):
                nc.tensor.transpose(psT[:, k], ht[:, k * P:(k + 1) * P], ident)
            hT = hpool.tile([P, 2, P], f32, tag="hT")
            nc.scalar.copy(hT, psT)
            pp = psum.tile([P, ODIM], f32, tag="pp")
            nc.tensor.matmul(pp, hT[:, 0], wT[:, 0], start=True, stop=False)
            nc.tensor.matmul(pp, hT[:, 1], wT[:, 1], start=False, stop=False)
            nc.tensor.matmul(pp, ones1, brow, start=False, stop=True)
            expt = usb[:, g]
            nc.scalar.activation(expt, pp, AF.Exp)
            # the HW Exp table is only ~1e-5 accurate; one Newton step off the
            # (much more accurate) Ln table: u1 = u0*(1 + (z - ln(u0)))
            lu = hpool.tile([P, ODIM], f32, tag="lu")
            nc.scalar.activation(lu, expt, AF.Ln)
            dz = hpool.tile([P, ODIM], f32, tag="dz")
            nc.vector.tensor_tensor(dz, pp, lu, OP.subtract)
            nc.vector.tensor_tensor(dz, expt, dz, OP.mult)
            nc.vector.tensor_tensor(expt, expt, dz, OP.add)
            # softplus: Ln(exp(z) + 1)
            nc.scalar.activation(params[:, g], expt, AF.Ln, bias=1.0, scale=1.0)

        # Ln(1+u) via table floors at ~ulp(1) absolute, which is a large
        # relative error when softplus(z) ~ 1e-6 (it then competes with the
        # +1e-4 epsilon).  For u < 1/16 use the log1p series directly.
        u2sb = bs.tile([P, G, ODIM], f32, tag="u2sb")
        nc.vector.tensor_tensor(u2sb, usb, usb, OP.mult)
        pA = bs.tile([P, G, ODIM], f32, tag="pA")
        nc.vector.tensor_scalar(pA, usb, -0.5, 1.0, op0=OP.mult, op1=OP.add)
        pB = bs.tile([P, G, ODIM], f32, tag="pB")
        nc.vector.tensor_scalar(pB, usb, -0.25, 1.0 / 3, op0=OP.mult, op1=OP.add)
        pC = bs.tile([P, G, ODIM], f32, tag="pC")
        nc.vector.tensor_scalar(pC, usb, -1.0 / 6, 0.2, op0=OP.mult, op1=OP.add)
        u4sb = bs.tile([P, G, ODIM], f32, tag="u4sb")
        nc.vector.tensor_tensor(u4sb, u2sb, u2sb, OP.mult)
        nc.vector.tensor_tensor(pB, pB, u2sb, OP.mult)
        nc.vector.tensor_tensor(pA, pA, pB, OP.add)
        nc.vector.tensor_tensor(pC, pC, u4sb, OP.mult)
        nc.vector.tensor_tensor(pA, pA, pC, OP.add)
        nc.vector.tensor_tensor(pA, pA, usb, OP.mult)
        msk = bs.tile([P, G, ODIM], mybir.dt.uint8, tag="msk")
        nc.vector.tensor_scalar(msk, usb, 0.0625, None, op0=OP.is_lt)
        nc.vector.copy_predicated(params, msk, pA)

        # ---------------- phase 2: bin-space coefficients -------------------
        w_in = bs.tile([P, G, NK], f32, tag="w_in")
        nc.vector.tensor_scalar(w_in, params[:, :, 0:NK], 1e-4, None, op0=OP.add)
        h_in = bs.tile([P, G, NK], f32, tag="h_in")
        nc.vector.tensor_scalar(h_in, params[:, :, NK:2 * NK], 1e-4, None, op0=OP.add)
        dpad = bs.tile([P, G, NK + 2], f32, tag="dpad")
        nc.vector.memset(dpad, 1.0)
        nc.vector.tensor_scalar(
            dpad[:, :, 1:NK + 1], params[:, :, 2 * NK:3 * NK], 1e-4, None, op0=OP.add)

        cx = bs.tile([P, G, NK], f32, tag="cx")
        nc.vector.tensor_tensor_scan(
            cx.rearrange("p g k -> p (g k)"),
            gate.rearrange("p g k -> p (g k)"),
            w_in.rearrange("p g k -> p (g k)"),
            0.0, op0=OP.mult, op1=OP.add)
        cy = bs.tile([P, G, NK], f32, tag="cy")
        nc.vector.tensor_tensor_scan(
            cy.rearrange("p g k -> p (g k)"),
            gate.rearrange("p g k -> p (g k)"),
            h_in.rearrange("p g k -> p (g k)"),
            0.0, op0=OP.mult, op1=OP.add)

        rw = bs.tile([P, G, NK], f32, tag="rw")
        nc.vector.reciprocal(rw, w_in)
        rsh = bs.tile([P, G], f32, tag="rsh")
        nc.vector.reciprocal(rsh, cy[:, :, NK - 1])
        lam = bs.tile([P, G], f32, tag="lam")
        nc.vector.tensor_tensor(lam, cx[:, :, NK - 1], rsh, OP.mult)

        # delta_hat = lam * h/w  (lam multiply done per-block: per-partition scalar)
        dd = bs.tile([P, G, NK], f32, tag="dd")
        nc.vector.tensor_tensor(dd, h_in, rw, OP.mult)
        for g in range(G):
            nc.vector.tensor_scalar(dd[:, g], dd[:, g], lam[:, g:g + 1], None,
                                    op0=OP.mult)

        dl = dpad[:, :, 0:NK]
        dr = dpad[:, :, 1:NK + 1]
        s_t = bs.tile([P, G, NK], f32, tag="s_t")
        nc.vector.tensor_tensor(s_t, dl, dr, OP.add)
        nc.vector.scalar_tensor_tensor(s_t, dd, -2.0, s_t, op0=OP.mult, op1=OP.add)

        apc = bs.tile([P, G, NK], f32, tag="apc")     # A' = h*(dd - dl)
        nc.vector.tensor_tensor(apc, dd, dl, OP.subtract)
        nc.vector.tensor_tensor(apc, apc, h_in, OP.mult)
        bpc = bs.tile([P, G, NK], f32, tag="bpc")     # B' = h*dl*w
        nc.vector.tensor_tensor(bpc, h_in, dl, OP.mult)
        nc.vector.tensor_tensor(bpc, bpc, w_in, OP.mult)

        w2 = bs.tile([P, G, NK], f32, tag="w2")
        nc.vector.tensor_tensor(w2, w_in, w_in, OP.mult)
        c0 = bs.tile([P, G, NK], f32, tag="c0")
        nc.vector.tensor_tensor(c0, dd, w2, OP.mult)
        c1 = bs.tile([P, G, NK], f32, tag="c1")
        nc.vector.tensor_tensor(c1, s_t, w_in, OP.mult)
        c2 = bs.tile([P, G, NK], f32, tag="c2")
        nc.vector.tensor_scalar(c2, s_t, -1.0, None, op0=OP.mult)

        # chain order: 0:a (jumps w) 1:cy (jumps h) 2:A' 3:B' 4:C0 5:C1 6:C2
        coefs = [None, None, apc, bpc, c0, c1, c2]
        dts = [w_in, h_in]
        for ci in range(2, 7):
            q = coefs[ci]
            dq = bs.tile([P, G, NK - 1], f32, tag=f"dq{ci}")
            nc.vector.tensor_tensor(dq, q[:, :, 1:NK], q[:, :, 0:NK - 1],
                                    OP.subtract)
            dts.append(dq)

        thr = bs.tile([P, G], f32, tag="thr")
        nc.vector.tensor_scalar(thr, cy[:, :, NK - 1], 1e-8, None, op0=OP.mult)

        # ---------------- phase 3: edge-space evaluation ---------------------
        for g in range(G):
            blk = sb * G + g
            r0 = blk * P
            sw = cx[:, g, NK - 1:NK]

            # Xt = x * Sw on ACT (per-partition scale)
            xt = es.tile([P, NE], f32, tag="xt")
            nc.scalar.activation(xt, xconst, AF.Copy, bias=0.0, scale=sw)

            # chain accumulators; init via fused tensor_scalar (2x_2p mode):
            #   chains 0,1 (a, cy): base 0, init = step j=1
            #   chains 2..6: init = base value ((Xt > -1) * base == base)
            accs = [es.tile([P, NE], f32, tag=f"acc{ci}", name=f"acc{ci}")
                    for ci in range(7)]
            nc.vector.tensor_scalar(accs[0], xt, cx[:, g, 0:1], w_in[:, g, 0:1],
                                    op0=OP.is_gt, op1=OP.mult)
            nc.vector.tensor_scalar(accs[1], xt, cx[:, g, 0:1], h_in[:, g, 0:1],
                                    op0=OP.is_gt, op1=OP.mult)
            for ci in range(2, 7):
                nc.vector.tensor_scalar(accs[ci], xt, -1.0,
                                        coefs[ci][:, g, 0:1],
                                        op0=OP.is_gt, op1=OP.mult)
            # steps j = 1..8 (threshold cx_j = cx[:, g, j-1]); chains 0,1
            # already consumed j=1 in their init.  j-major interleave keeps
            # consecutive DVE ops on different accumulators.
            for j in range(1, NK):
                cxs = cx[:, g, j - 1:j]
                for ci in range(7):
                    if ci < 2 and j == 1:
                        continue
                    step_madd(accs[ci], xt, cxs, dts[ci][:, g, j - 1:j])

            # u = xt - a ; u2 = u^2 (ACT Square)
            u = es.tile([P, NE], f32, tag="u")
            nc.gpsimd.tensor_tensor(u, xt, accs[0], OP.subtract)
            u2 = es.tile([P, NE], f32, tag="u2")
            nc.scalar.activation(u2, u, AF.Square)

            # numerator m = u*(A'*u + B')
            m = es.tile([P, NE], f32, tag="m")
            nc.gpsimd.tensor_tensor(m, accs[2], u, OP.mult)
            nc.gpsimd.tensor_tensor(m, m, accs[3], OP.add)
            nc.gpsimd.tensor_tensor(m, m, u, OP.mult)

            # denominator D = C0 + C1*u + C2*u^2
            d1 = es.tile([P, NE], f32, tag="d1")
            nc.gpsimd.tensor_tensor(d1, accs[5], u, OP.mult)
            nc.gpsimd.tensor_tensor(d1, d1, accs[4], OP.add)
            t3 = es.tile([P, NE], f32, tag="t3")
            nc.gpsimd.tensor_tensor(t3, accs[6], u2, OP.mult)
            nc.gpsimd.tensor_tensor(d1, d1, t3, OP.add)

            # rd = 1/D: DVE fast seed (~51 ULP) + one Newton step on GPSIMD
            y0 = es.tile([P, NE], f32, tag="y0")
            nc.vector.reciprocal_approx_fast(y0, d1)
            t4 = es.tile([P, NE], f32, tag="t4")
            nc.gpsimd.tensor_tensor(t4, d1, y0, OP.mult)
            nc.gpsimd.tensor_tensor(t4, two_const, t4, OP.subtract)
            nc.gpsimd.tensor_tensor(t4, y0, t4, OP.mult)
            rat = es.tile([P, NE], f32, tag="rat")
            nc.gpsimd.tensor_tensor(rat, m, t4, OP.mult)

            # probs~ = diff(cy_chain) + diff(rat); within-bin cy diffs cancel
            # bit-exactly, keeping the difference at increment scale.
            pt = outp.tile([P, NB], f32, tag="pt")
            nc.gpsimd.tensor_tensor(pt, accs[1][:, 1:NE], accs[1][:, 0:NB],
                                    OP.subtract)
            dr_ = outp.tile([P, NB], f32, tag="dr")
            nc.gpsimd.tensor_tensor(dr_, rat[:, 1:NE], rat[:, 0:NB],
                                    OP.subtract)
            nc.gpsimd.tensor_tensor(pt, pt, dr_, OP.add)
            nc.vector.tensor_scalar(pt, pt, thr[:, g:g + 1], None, op0=OP.max)
            ot = outp.tile([P, NB], f32, tag="ot")
            nc.scalar.activation(ot, pt, AF.Ln, bias=0.0, scale=rsh[:, g:g + 1])
            nc.sync.dma_start(out=out[r0:r0 + P, :], in_=ot)


def make_nc(rows, gpsimd_chains=None):
    import concourse.bacc as bacc
    nc = bacc.Bacc("TRN2", target_bir_lowering=False, debug=False,
                   num_devices=N_CORES)
    h_t = nc.dram_tensor("h", [rows, IN_DIM], f32, kind="ExternalInput").ap()
    W_t = nc.dram_tensor("W", [ODIM, IN_DIM], f32, kind="ExternalInput").ap()
    b_t = nc.dram_tensor("b", [ODIM], f32, kind="ExternalInput").ap()
    out_t = nc.dram_tensor("out", [rows, NB], f32, kind="ExternalOutput").ap()
    with tile.TileContext(nc) as tc:
        with ExitStack() as ctx:
            build_rqs(ctx, tc, h_t, W_t, b_t, out_t, rows)
    nc.compile()
    return nc


_cache = {}


def kernel(h, W, b):
    h = np.ascontiguousarray(h, dtype=np.float32)
    W = np.ascontiguousarray(W, dtype=np.float32)
    b = np.ascontiguousarray(b, dtype=np.float32)
    rows = h.shape[0] // N_CORES
    key = ("nc", rows)
    if key not in _cache:
        _cache[key] = make_nc(rows)
    nc = _cache[key]
    from concourse.bass_utils import run_bass_kernel_spmd
    in_maps = [
        {"h": h[i * rows:(i + 1) * rows], "W": W, "b": b}
        for i in range(N_CORES)
    ]
    res = run_bass_kernel_spmd(nc, in_maps, core_ids=list(range(N_CORES)))
    return np.concatenate([r["out"] for r in res.results], axis=0)


if __name__ == "__main__":
    rng = np.random.default_rng(0)
    h = rng.standard_normal((B_FULL, IN_DIM), dtype=np.float32)
    W = (rng.standard_normal((ODIM, IN_DIM), dtype=np.float32) / 16.0)
    b = rng.standard_normal((ODIM,), dtype=np.float32) * 0.01
    out = kernel(h, W, b)
    print(out.shape, out.dtype, out[:2, :4])


# revision 4
# speedup vs baseline: 2.6887x; 2.4328x over previous
"""RQSplineHead Trainium2 Bass kernel (custom fused DVE chain ops).

Same math as kernel.py (RQS CDF histogram, 7 step-mask accumulation
chains per 128-row block), but the chain work is split across engines:

  - DVE blocks: chains via the custom fused DVE op STEP_MADD_ANT
    (out = in1 + (in0 > s0)*s1), one instruction per (chain, knot).
  - GPSIMD blocks: exact 0/1 masks built on ACT (Relu then Sign), then
    all 7 chains accumulated together with chain-batched [P, 7, NE]
    tensor_tensor passes using stride-0 broadcast operands (mask along
    chains, dq along edges).  2 passes per knot for all 7 chains.
  - The rational evaluation is batched over groups of 4 blocks
    ([P, 4, NE] tensor ops on DVE), amortizing per-instruction fixed
    cost; the final Ln runs on ACT as one [P, 4, 128] op after probs
    are pre-scaled by 1/Sh (so the clamp threshold is a constant 1e-8).

The DVE/GPSIMD block ratio is set by GP_NUM/GP_DEN.
"""

import numpy as np
from contextlib import ExitStack

import concourse.bass as bass
import concourse.mybir as mybir
import concourse.tile as tile
from concourse.masks import make_identity

f32 = mybir.dt.float32
i32 = mybir.dt.int32
OP = mybir.AluOpType
AF = mybir.ActivationFunctionType

B_FULL = 131072
IN_DIM = 256
NE = 129          # edges (n_bins + 1)
NB = 128          # output bins
NK = 9            # spline bins per row (n_knots + 1)
NCH = 7           # piecewise chains
ODIM = 27
N_CORES = 8
P = 128
B4 = 4            # eval batch (blocks per batched rational evaluation)

# fraction of blocks whose chains run on GPSIMD: GP_NUM / GP_DEN
GP_NUM = 0
GP_DEN = 64
FANCY_OUT_DMA = True
BASES_IN1 = False     # [P,1] in1 on the custom op crashes the HW exec unit
BATCHED_EVAL = True   # batched [P,B4,NE] eval vs per-block eval
EVAL_NR_ON_GP = False  # Newton step of 1/D on GPSIMD vs DVE


def _make_step_op():
    from concourse import dve_ops as DO
    from concourse import dve_spec as DS
    from concourse.dve_uop import DveOpSpec

    name = "STEP_MADD_ANT"
    for op in DO.OPS:
        if op.name == name:
            return op
    spec = DS.Spec(
        body=DS.Src1 + (DS.Src0 > DS.C0) * DS.C1,
        reference=lambda in0, in1, s0, s1, imm2: in1 + (in0 > s0) * s1,
    )
    row = max(DO._SUB_OPCODE_FOR_NAME.values()) + 1
    assert row < 0x20, "no free custom-DVE opcode rows"
    DO._SUB_OPCODE_FOR_NAME[name] = row
    shas = {}
    for ver in ("v3", "v4"):
        try:
            s = DveOpSpec(name=name, opcode=row,
                          uops=DS.lower(spec, ver=ver), rd1_en=True)
            shas[ver] = s.sha(ver)
        except Exception:
            pass
    assert shas, "failed to lower STEP_MADD_ANT for any DVE version"
    op = DO.DveOp(name, spec, subdim=False, uops_sha=shas)
    DO.OPS.append(op)
    DO.CUSTOM_DVE_SPECS[name] = spec
    return op


STEP_MADD = _make_step_op()


def _bc(ap, shape, tag):
    """stride-0 broadcast helper: expand a missing middle/inner dim."""
    return ap.broadcast_to(shape) if list(ap.shape) == list(shape) else ap


def build_rqs(ctx: ExitStack, tc: "tile.TileContext", h, W, b, out, rows):
    nc = tc.nc
    nblk = rows // P
    G = min(8, nblk)
    nsb = nblk // G
    assert nsb * G == nblk, (rows, nblk, G)
    assert G % B4 == 0

    def step_madd(acc, xt, cxs, dq, in1=None):
        nc.vector._custom_dve(STEP_MADD, out=acc, in0=xt,
                              in1=(acc if in1 is None else in1),
                              s0=cxs, s1=dq)

    const = ctx.enter_context(tc.tile_pool(name="const", bufs=1))
    psum = ctx.enter_context(tc.tile_pool(name="psum", bufs=2, space="PSUM"))
    psum1 = ctx.enter_context(tc.tile_pool(name="psum1", bufs=1, space="PSUM"))
    hpool = ctx.enter_context(tc.tile_pool(name="hpool", bufs=3))
    bs = ctx.enter_context(tc.tile_pool(name="bs", bufs=2))
    es = ctx.enter_context(tc.tile_pool(name="es", bufs=4))
    ev = ctx.enter_context(tc.tile_pool(name="ev", bufs=3))
    outp = ctx.enter_context(tc.tile_pool(name="outp", bufs=2))

    # ---------------- constants ----------------
    ident = const.tile([P, P], f32)
    make_identity(nc, ident)

    xi = const.tile([P, NE], i32)
    nc.gpsimd.iota(xi, pattern=[[1, NE]], base=0, channel_multiplier=0)
    xconst = const.tile([P, NE], f32)
    nc.vector.tensor_scalar(xconst, xi, 1.0 / NB, None, op0=OP.mult)

    gate = const.tile([P, G, NK], f32)
    nc.vector.memset(gate, 1.0)
    nc.vector.memset(gate[:, :, 0:1], 0.0)

    ones1 = const.tile([1, P], f32)
    nc.vector.memset(ones1, 1.0)

    two4 = const.tile([P, B4, NE], f32)
    nc.vector.memset(two4, 2.0)

    onesNB = const.tile([P, NB], f32)
    nc.vector.memset(onesNB, 1.0)

    wraw = const.tile([P, IN_DIM], f32)
    nc.vector.memset(wraw, 0.0)
    nc.sync.dma_start(out=wraw[0:ODIM, :], in_=W)
    psw = psum1.tile([P, 2, P], f32)
    for k in range(2):
        nc.tensor.transpose(psw[:, k], wraw[:, k * P:(k + 1) * P], ident)
    wT = const.tile([P, 2, ODIM], f32)
    nc.scalar.copy(wT, psw[:, :, 0:ODIM])
    brow = const.tile([1, ODIM], f32)
    nc.sync.dma_start(out=brow, in_=b.rearrange("(o k) -> o k", o=1))

    pending_eval = []

    def emit_eval(acc4, xt4, rsh_aps, r0):
        aA = acc4[:, :, 0]
        aCY = acc4[:, :, 1]
        aAp = acc4[:, :, 2]
        aBp = acc4[:, :, 3]
        aC0 = acc4[:, :, 4]
        aC1 = acc4[:, :, 5]
        aC2 = acc4[:, :, 6]

        u = ev.tile([P, B4, NE], f32, tag="u", name="u")
        nc.vector.tensor_tensor(u, xt4, aA, OP.subtract)
        u2 = ev.tile([P, B4, NE], f32, tag="u2", name="u2")
        nc.scalar.activation(u2, u, AF.Square)

        m = ev.tile([P, B4, NE], f32, tag="m", name="m")
        nc.vector.tensor_tensor(m, aAp, u, OP.mult)
        nc.vector.tensor_tensor(m, m, aBp, OP.add)
        nc.vector.tensor_tensor(m, m, u, OP.mult)

        d1 = ev.tile([P, B4, NE], f32, tag="d1", name="d1")
        nc.vector.tensor_tensor(d1, aC1, u, OP.mult)
        nc.vector.tensor_tensor(d1, d1, aC0, OP.add)
        t3 = ev.tile([P, B4, NE], f32, tag="t3", name="t3")
        nc.vector.tensor_tensor(t3, aC2, u2, OP.mult)
        nc.vector.tensor_tensor(d1, d1, t3, OP.add)

        y0 = ev.tile([P, B4, NE], f32, tag="y0", name="y0")
        nc.vector.reciprocal_approx_fast(y0, d1)
        t4 = ev.tile([P, B4, NE], f32, tag="t4", name="t4")
        eng = nc.gpsimd if EVAL_NR_ON_GP else nc.vector
        eng.tensor_tensor(t4, d1, y0, OP.mult)
        eng.tensor_tensor(t4, two4, t4, OP.subtract)
        eng.tensor_tensor(t4, y0, t4, OP.mult)
        rat = ev.tile([P, B4, NE], f32, tag="rat", name="rat")
        nc.vector.tensor_tensor(rat, m, t4, OP.mult)

        pt = outp.tile([P, B4, NB], f32, tag="pt")
        nc.vector.tensor_tensor(pt, aCY[:, :, 1:NE], aCY[:, :, 0:NB],
                                OP.subtract)
        dr_ = outp.tile([P, B4, NB], f32, tag="dr")
        nc.vector.tensor_tensor(dr_, rat[:, :, 1:NE], rat[:, :, 0:NB],
                                OP.subtract)
        nc.vector.tensor_tensor(pt, pt, dr_, OP.add)
        # scale by 1/Sh first so the clamp threshold is constant 1e-8
        # (rsh broadcast materialized on ACT; stride-0 TT operands are
        # not HW-safe on DVE)
        rshT = outp.tile([P, B4, NB], f32, tag="rshT")
        for bb in range(B4):
            nc.scalar.activation(rshT[:, bb], onesNB, AF.Copy,
                                 bias=0.0, scale=rsh_aps[bb])
        nc.vector.tensor_tensor(pt, pt, rshT, OP.mult)
        nc.vector.tensor_scalar(pt, pt, 1e-8, None, op0=OP.max)
        ot = outp.tile([P, B4, NB], f32, tag="ot")
        nc.scalar.activation(ot, pt, AF.Ln)
        if FANCY_OUT_DMA:
            out_view = out[r0:r0 + B4 * P, :].rearrange(
                "(b p) n -> p b n", b=B4)
            nc.sync.dma_start(out=out_view, in_=ot)
        else:
            for bb in range(B4):
                rb = r0 + bb * P
                nc.sync.dma_start(out=out[rb:rb + P, :], in_=ot[:, bb])

    for sb in range(nsb):
        # ---------------- phase 1: params = softplus(h @ W.T + b) ----------
        params = bs.tile([P, G, ODIM], f32, tag="params")
        usb = bs.tile([P, G, ODIM], f32, tag="usb")
        for g in range(G):
            blk = sb * G + g
            r0 = blk * P
            ht = hpool.tile([P, IN_DIM], f32, tag="ht")
            nc.sync.dma_start(out=ht, in_=h[r0:r0 + P, :])
            psT = psum.tile([P, 2, P], f32, tag="psT")
            for k in range(2):
                nc.tensor.transpose(psT[:, k], ht[:, k * P:(k + 1) * P], ident)
            hT = hpool.tile([P, 2, P], f32, tag="hT")
            nc.scalar.copy(hT, psT)
            pp = psum.tile([P, ODIM], f32, tag="pp")
            nc.tensor.matmul(pp, hT[:, 0], wT[:, 0], start=True, stop=False)
            nc.tensor.matmul(pp, hT[:, 1], wT[:, 1], start=False, stop=False)
            nc.tensor.matmul(pp, ones1, brow, start=False, stop=True)
            expt = usb[:, g]
            nc.scalar.activation(expt, pp, AF.Exp)
            lu = hpool.tile([P, ODIM], f32, tag="lu")
            nc.scalar.activation(lu, expt, AF.Ln)
            dz = hpool.tile([P, ODIM], f32, tag="dz")
            nc.vector.tensor_tensor(dz, pp, lu, OP.subtract)
            nc.vector.tensor_tensor(dz, expt, dz, OP.mult)
            nc.vector.tensor_tensor(expt, expt, dz, OP.add)
            nc.scalar.activation(params[:, g], expt, AF.Ln, bias=1.0, scale=1.0)

        # log1p series fixup for small softplus outputs
        u2sb = bs.tile([P, G, ODIM], f32, tag="u2sb")
        nc.vector.tensor_tensor(u2sb, usb, usb, OP.mult)
        pA = bs.tile([P, G, ODIM], f32, tag="pA")
        nc.vector.tensor_scalar(pA, usb, -0.5, 1.0, op0=OP.mult, op1=OP.add)
        pB = bs.tile([P, G, ODIM], f32, tag="pB")
        nc.vector.tensor_scalar(pB, usb, -0.25, 1.0 / 3, op0=OP.mult, op1=OP.add)
        pC = bs.tile([P, G, ODIM], f32, tag="pC")
        nc.vector.tensor_scalar(pC, usb, -1.0 / 6, 0.2, op0=OP.mult, op1=OP.add)
        u4sb = bs.tile([P, G, ODIM], f32, tag="u4sb")
        nc.vector.tensor_tensor(u4sb, u2sb, u2sb, OP.mult)
        nc.vector.tensor_tensor(pB, pB, u2sb, OP.mult)
        nc.vector.tensor_tensor(pA, pA, pB, OP.add)
        nc.vector.tensor_tensor(pC, pC, u4sb, OP.mult)
        nc.vector.tensor_tensor(pA, pA, pC, OP.add)
        nc.vector.tensor_tensor(pA, pA, usb, OP.mult)
        msk = bs.tile([P, G, ODIM], mybir.dt.uint8, tag="msk")
        nc.vector.tensor_scalar(msk, usb, 0.0625, None, op0=OP.is_lt)
        nc.vector.copy_predicated(params, msk, pA)

        # ---------------- phase 2: bin-space coefficients -------------------
        w_in = bs.tile([P, G, NK], f32, tag="w_in")
        nc.vector.tensor_scalar(w_in, params[:, :, 0:NK], 1e-4, None, op0=OP.add)
        h_in = bs.tile([P, G, NK], f32, tag="h_in")
        nc.vector.tensor_scalar(h_in, params[:, :, NK:2 * NK], 1e-4, None, op0=OP.add)
        dpad = bs.tile([P, G, NK + 2], f32, tag="dpad")
        nc.vector.memset(dpad, 1.0)
        nc.vector.tensor_scalar(
            dpad[:, :, 1:NK + 1], params[:, :, 2 * NK:3 * NK], 1e-4, None, op0=OP.add)

        cx = bs.tile([P, G, NK], f32, tag="cx")
        nc.vector.tensor_tensor_scan(
            cx.rearrange("p g k -> p (g k)"),
            gate.rearrange("p g k -> p (g k)"),
            w_in.rearrange("p g k -> p (g k)"),
            0.0, op0=OP.mult, op1=OP.add)
        cy = bs.tile([P, G, NK], f32, tag="cy")
        nc.vector.tensor_tensor_scan(
            cy.rearrange("p g k -> p (g k)"),
            gate.rearrange("p g k -> p (g k)"),
            h_in.rearrange("p g k -> p (g k)"),
            0.0, op0=OP.mult, op1=OP.add)

        # negated thresholds for ACT mask bias: -cx_j
        ncx = bs.tile([P, G, NK - 1], f32, tag="ncx")
        nc.vector.tensor_scalar(ncx, cx[:, :, 0:NK - 1], -1.0, None, op0=OP.mult)

        rw = bs.tile([P, G, NK], f32, tag="rw")
        nc.vector.reciprocal(rw, w_in)
        rsh = bs.tile([P, G], f32, tag="rsh")
        nc.vector.reciprocal(rsh, cy[:, :, NK - 1])
        lam = bs.tile([P, G], f32, tag="lam")
        nc.vector.tensor_tensor(lam, cx[:, :, NK - 1], rsh, OP.mult)

        dd = bs.tile([P, G, NK], f32, tag="dd")
        nc.vector.tensor_tensor(dd, h_in, rw, OP.mult)
        for g in range(G):
            nc.vector.tensor_scalar(dd[:, g], dd[:, g], lam[:, g:g + 1], None,
                                    op0=OP.mult)

        dl = dpad[:, :, 0:NK]
        dr = dpad[:, :, 1:NK + 1]
        s_t = bs.tile([P, G, NK], f32, tag="s_t")
        nc.vector.tensor_tensor(s_t, dl, dr, OP.add)
        nc.vector.scalar_tensor_tensor(s_t, dd, -2.0, s_t, op0=OP.mult, op1=OP.add)

        apc = bs.tile([P, G, NK], f32, tag="apc")     # A' = h*(dd - dl)
        nc.vector.tensor_tensor(apc, dd, dl, OP.subtract)
        nc.vector.tensor_tensor(apc, apc, h_in, OP.mult)
        bpc = bs.tile([P, G, NK], f32, tag="bpc")     # B' = h*dl*w
        nc.vector.tensor_tensor(bpc, h_in, dl, OP.mult)
        nc.vector.tensor_tensor(bpc, bpc, w_in, OP.mult)

        w2 = bs.tile([P, G, NK], f32, tag="w2")
        nc.vector.tensor_tensor(w2, w_in, w_in, OP.mult)
        c0 = bs.tile([P, G, NK], f32, tag="c0")
        nc.vector.tensor_tensor(c0, dd, w2, OP.mult)
        c1 = bs.tile([P, G, NK], f32, tag="c1")
        nc.vector.tensor_tensor(c1, s_t, w_in, OP.mult)
        c2 = bs.tile([P, G, NK], f32, tag="c2")
        nc.vector.tensor_scalar(c2, s_t, -1.0, None, op0=OP.mult)

        # chain tables: dq_all[p, g, ci, j] (jump at knot j+1), bases[p, g, ci]
        coefs = [None, None, apc, bpc, c0, c1, c2]
        dq_all = bs.tile([P, G, NCH, NK - 1], f32, tag="dq_all")
        nc.vector.tensor_scalar(dq_all[:, :, 0], w_in[:, :, 0:NK - 1], 1.0,
                                None, op0=OP.mult)
        nc.vector.tensor_scalar(dq_all[:, :, 1], h_in[:, :, 0:NK - 1], 1.0,
                                None, op0=OP.mult)
        for ci in range(2, NCH):
            q = coefs[ci]
            nc.vector.tensor_tensor(dq_all[:, :, ci], q[:, :, 1:NK],
                                    q[:, :, 0:NK - 1], OP.subtract)
        bases = bs.tile([P, G, NCH], f32, tag="bases")
        nc.vector.memset(bases[:, :, 0:2], 0.0)
        for ci in range(2, NCH):
            nc.vector.tensor_scalar(bases[:, :, ci:ci + 1],
                                    coefs[ci][:, :, 0:1], 1.0, None,
                                    op0=OP.mult)

        # ---------------- phase 3: chains + batched evaluation --------------
        # Software pipeline: the (DVE-heavy) eval of group k is emitted
        # after the chains of group k+1, so DVE keeps chain work in
        # flight while GPSIMD finishes its blocks of group k.
        for g4 in range(G // B4):
            acc4 = ev.tile([P, B4, NCH, NE], f32, tag="acc4", name="acc4")
            xt4 = ev.tile([P, B4, NE], f32, tag="xt4", name="xt4")
            gblocks = []
            dblocks = []
            for bb in range(B4):
                g = g4 * B4 + bb
                blk = sb * G + g
                sw = cx[:, g, NK - 1:NK]
                nc.scalar.activation(xt4[:, bb], xconst, AF.Copy, bias=0.0,
                                     scale=sw)
                on_gp = (blk * GP_NUM) % GP_DEN < GP_NUM
                (gblocks if on_gp else dblocks).append((bb, g))
            for bb, g in gblocks + dblocks:
                xt = xt4[:, bb]
                acc = acc4[:, bb]
                on_gp = (bb, g) in gblocks

                if not on_gp:
                    # DVE path: custom fused step ops, j-major interleave
                    nc.vector.tensor_scalar(
                        acc[:, 0], xt, cx[:, g, 0:1], w_in[:, g, 0:1],
                        op0=OP.is_gt, op1=OP.mult)
                    nc.vector.tensor_scalar(
                        acc[:, 1], xt, cx[:, g, 0:1], h_in[:, g, 0:1],
                        op0=OP.is_gt, op1=OP.mult)
                    # chains 2..6 init = base value, broadcast on ACT
                    bview = bases[:, g, 2:NCH].rearrange(
                        "p (c e) -> p c e", e=1).broadcast_to(
                        [P, NCH - 2, NE])
                    nc.scalar.activation(acc[:, 2:NCH], bview, AF.Copy)
                    for ci in range(2, NCH):
                        step_madd(acc[:, ci], xt, cx[:, g, 0:1],
                                  dq_all[:, g, ci, 0:1])
                    for j in range(2, NK):
                        cxs = cx[:, g, j - 1:j]
                        for ci in range(NCH):
                            step_madd(acc[:, ci], xt, cxs,
                                      dq_all[:, g, ci, j - 1:j])
                else:
                    # GPSIMD path: exact masks on ACT, chain-batched madd
                    for j in range(1, NK):
                        mask = es.tile([P, NE], f32, tag="mask", name="mask")
                        nc.scalar.activation(mask, xt, AF.Relu,
                                             bias=ncx[:, g, j - 1:j], scale=1.0)
                        nc.scalar.activation(mask, mask, AF.Sign)
                        mask_b = mask.rearrange(
                            "p (c e) -> p c e", c=1).broadcast_to([P, NCH, NE])
                        dq_b = dq_all[:, g, :, j - 1].rearrange(
                            "p (c e) -> p c e", e=1).broadcast_to([P, NCH, NE])
                        if j == 1:
                            nc.gpsimd.tensor_tensor(acc, mask_b, dq_b, OP.mult)
                        else:
                            term = es.tile([P, NCH, NE], f32, tag="term",
                                           name="term")
                            nc.gpsimd.tensor_tensor(term, mask_b, dq_b, OP.mult)
                            nc.gpsimd.tensor_tensor(acc, acc, term, OP.add)
                    bases_b = bases[:, g].rearrange(
                        "p (c e) -> p c e", e=1).broadcast_to([P, NCH, NE])
                    nc.gpsimd.tensor_tensor(acc, acc, bases_b, OP.add)

            # ---- batched rational evaluation over B4 blocks (DVE) ----
            g0 = g4 * B4
            blk0 = sb * G + g0
            rsh_aps = [rsh[:, g0 + bb:g0 + bb + 1] for bb in range(B4)]
            if pending_eval:
                pending_eval.pop(0)()
            pending_eval.append(
                (lambda a4=acc4, x4=xt4, ra=rsh_aps, r0=blk0 * P:
                 emit_eval(a4, x4, ra, r0)))

    while pending_eval:
        pending_eval.pop(0)()


def make_nc(rows, gpsimd_chains=None):
    import concourse.bacc as bacc
    nc = bacc.Bacc("TRN2", target_bir_lowering=False, debug=False,
                   num_devices=N_CORES)
    h_t = nc.dram_tensor("h", [rows, IN_DIM], f32, kind="ExternalInput").ap()
    W_t = nc.dram_tensor("W", [ODIM, IN_DIM], f32, kind="ExternalInput").ap()
    b_t = nc.dram_tensor("b", [ODIM], f32, kind="ExternalInput").ap()
    out_t = nc.dram_tensor("out", [rows, NB], f32, kind="ExternalOutput").ap()
    with tile.TileContext(nc) as tc:
        with ExitStack() as ctx:
            build_rqs(ctx, tc, h_t, W_t, b_t, out_t, rows)
    nc.compile()
    return nc


_cache = {}


def kernel(h, W, b):
    h = np.ascontiguousarray(h, dtype=np.float32)
    W = np.ascontiguousarray(W, dtype=np.float32)
    b = np.ascontiguousarray(b, dtype=np.float32)
    rows = h.shape[0] // N_CORES
    key = ("nc", rows)
    if key not in _cache:
        _cache[key] = make_nc(rows)
    nc = _cache[key]
    from concourse.bass_utils import run_bass_kernel_spmd
    in_maps = [
        {"h": h[i * rows:(i + 1) * rows], "W": W, "b": b}
        for i in range(N_CORES)
    ]
    res = run_bass_kernel_spmd(nc, in_maps, core_ids=list(range(N_CORES)))
    return np.concatenate([r["out"] for r in res.results], axis=0)


if __name__ == "__main__":
    rng = np.random.default_rng(0)
    h = rng.standard_normal((B_FULL, IN_DIM), dtype=np.float32)
    W = (rng.standard_normal((ODIM, IN_DIM), dtype=np.float32) / 16.0)
    b = rng.standard_normal((ODIM,), dtype=np.float32) * 0.01
    out = kernel(h, W, b)
    print(out.shape, out.dtype, out[:2, :4])


# revision 6
# speedup vs baseline: 2.7659x; 1.0287x over previous
"""RQSplineHead Trainium2 Bass kernel — hybrid chain engine version.

Same math as kernel.py (RQS CDF histogram, 7 step-mask accumulation
chains per 128-row block), but the chain work is split across engines:

  - DVE blocks: chains via the custom fused DVE op STEP_MADD_ANT
    (out = in1 + (in0 > s0)*s1), one instruction per (chain, knot).
  - GPSIMD blocks: exact 0/1 masks built on ACT (Relu then Sign), then
    all 7 chains accumulated together with chain-batched [P, 7, NE]
    tensor_tensor passes using stride-0 broadcast operands (mask along
    chains, dq along edges).  2 passes per knot for all 7 chains.
  - The rational evaluation is batched over groups of 4 blocks
    ([P, 4, NE] tensor ops on DVE), amortizing per-instruction fixed
    cost; the final Ln runs on ACT as one [P, 4, 128] op after probs
    are pre-scaled by 1/Sh (so the clamp threshold is a constant 1e-8).

The DVE/GPSIMD block ratio is set by GP_NUM/GP_DEN.
"""

import numpy as np
from contextlib import ExitStack

import concourse.bass as bass
import concourse.mybir as mybir
import concourse.tile as tile
from concourse.masks import make_identity

f32 = mybir.dt.float32
i32 = mybir.dt.int32
OP = mybir.AluOpType
AF = mybir.ActivationFunctionType

B_FULL = 131072
IN_DIM = 256
NE = 129          # edges (n_bins + 1)
NB = 128          # output bins
NK = 9            # spline bins per row (n_knots + 1)
NCH = 7           # piecewise chains
ODIM = 27
N_CORES = 8
P = 128
B4 = 8            # eval batch (blocks per batched rational evaluation)

# fraction of blocks whose chains run on GPSIMD: GP_NUM / GP_DEN
GP_NUM = 0
GP_DEN = 64
FANCY_OUT_DMA = True
BASES_IN1 = False     # [P,1] in1 on the custom op crashes the HW exec unit
BATCHED_EVAL = True   # batched [P,B4,NE] eval vs per-block eval
EVAL_NR_ON_GP = False  # Newton step of 1/D on GPSIMD vs DVE


def _make_step_op():
    from concourse import dve_ops as DO
    from concourse import dve_spec as DS
    from concourse.dve_uop import DveOpSpec

    name = "STEP_MADD_ANT"
    for op in DO.OPS:
        if op.name == name:
            return op
    spec = DS.Spec(
        body=DS.Src1 + (DS.Src0 > DS.C0) * DS.C1,
        reference=lambda in0, in1, s0, s1, imm2: in1 + (in0 > s0) * s1,
    )
    row = max(DO._SUB_OPCODE_FOR_NAME.values()) + 1
    assert row < 0x20, "no free custom-DVE opcode rows"
    DO._SUB_OPCODE_FOR_NAME[name] = row
    shas = {}
    for ver in ("v3", "v4"):
        try:
            s = DveOpSpec(name=name, opcode=row,
                          uops=DS.lower(spec, ver=ver), rd1_en=True)
            shas[ver] = s.sha(ver)
        except Exception:
            pass
    assert shas, "failed to lower STEP_MADD_ANT for any DVE version"
    op = DO.DveOp(name, spec, subdim=False, uops_sha=shas)
    DO.OPS.append(op)
    DO.CUSTOM_DVE_SPECS[name] = spec
    return op


STEP_MADD = _make_step_op()


def _bc(ap, shape, tag):
    """stride-0 broadcast helper: expand a missing middle/inner dim."""
    return ap.broadcast_to(shape) if list(ap.shape) == list(shape) else ap


def build_rqs(ctx: ExitStack, tc: "tile.TileContext", h, W, b, out, rows):
    nc = tc.nc
    nblk = rows // P
    G = min(8, nblk)
    nsb = nblk // G
    assert nsb * G == nblk, (rows, nblk, G)
    assert G % B4 == 0

    def step_madd(acc, xt, cxs, dq, in1=None):
        nc.vector._custom_dve(STEP_MADD, out=acc, in0=xt,
                              in1=(acc if in1 is None else in1),
                              s0=cxs, s1=dq)

    const = ctx.enter_context(tc.tile_pool(name="const", bufs=1))
    psum = ctx.enter_context(tc.tile_pool(name="psum", bufs=2, space="PSUM"))
    psum1 = ctx.enter_context(tc.tile_pool(name="psum1", bufs=1, space="PSUM"))
    hpool = ctx.enter_context(tc.tile_pool(name="hpool", bufs=3))
    bs = ctx.enter_context(tc.tile_pool(name="bs", bufs=2))
    es = ctx.enter_context(tc.tile_pool(name="es", bufs=4))
    ev = ctx.enter_context(tc.tile_pool(name="ev", bufs=2))
    et = ctx.enter_context(tc.tile_pool(name="et", bufs=2))
    outp = ctx.enter_context(tc.tile_pool(name="outp", bufs=2))

    # ---------------- constants ----------------
    ident = const.tile([P, P], f32)
    make_identity(nc, ident)

    xi = const.tile([P, NE], i32)
    nc.gpsimd.iota(xi, pattern=[[1, NE]], base=0, channel_multiplier=0)
    xconst = const.tile([P, NE], f32)
    nc.vector.tensor_scalar(xconst, xi, 1.0 / NB, None, op0=OP.mult)

    gate = const.tile([P, G, NK], f32)
    nc.vector.memset(gate, 1.0)
    nc.vector.memset(gate[:, :, 0:1], 0.0)

    ones1 = const.tile([1, P], f32)
    nc.vector.memset(ones1, 1.0)

    two4 = const.tile([P, B4, NE], f32)
    nc.vector.memset(two4, 2.0)

    onesNB = const.tile([P, NB], f32)
    nc.vector.memset(onesNB, 1.0)

    wraw = const.tile([P, IN_DIM], f32)
    nc.vector.memset(wraw, 0.0)
    nc.sync.dma_start(out=wraw[0:ODIM, :], in_=W)
    psw = psum1.tile([P, 2, P], f32)
    for k in range(2):
        nc.tensor.transpose(psw[:, k], wraw[:, k * P:(k + 1) * P], ident)
    wT = const.tile([P, 2, ODIM], f32)
    nc.scalar.copy(wT, psw[:, :, 0:ODIM])
    brow = const.tile([1, ODIM], f32)
    nc.sync.dma_start(out=brow, in_=b.rearrange("(o k) -> o k", o=1))

    pending_eval = []

    def emit_eval(acc4, xt4, rsh_aps, r0):
        aA = acc4[:, :, 0]
        aCY = acc4[:, :, 1]
        aAp = acc4[:, :, 2]
        aBp = acc4[:, :, 3]
        aC0 = acc4[:, :, 4]
        aC1 = acc4[:, :, 5]
        aC2 = acc4[:, :, 6]

        u = et.tile([P, B4, NE], f32, tag="u", name="u")
        nc.vector.tensor_tensor(u, xt4, aA, OP.subtract)
        u2 = et.tile([P, B4, NE], f32, tag="u2", name="u2")
        nc.scalar.activation(u2, u, AF.Square)

        m = et.tile([P, B4, NE], f32, tag="m", name="m")
        nc.vector.tensor_tensor(m, aAp, u, OP.mult)
        nc.vector.tensor_tensor(m, m, aBp, OP.add)
        nc.vector.tensor_tensor(m, m, u, OP.mult)

        d1 = et.tile([P, B4, NE], f32, tag="d1", name="d1")
        nc.vector.tensor_tensor(d1, aC1, u, OP.mult)
        nc.vector.tensor_tensor(d1, d1, aC0, OP.add)
        t3 = et.tile([P, B4, NE], f32, tag="t3", name="t3")
        nc.vector.tensor_tensor(t3, aC2, u2, OP.mult)
        nc.vector.tensor_tensor(d1, d1, t3, OP.add)

        y0 = et.tile([P, B4, NE], f32, tag="y0", name="y0")
        nc.vector.reciprocal_approx_fast(y0, d1)
        t4 = et.tile([P, B4, NE], f32, tag="t4", name="t4")
        eng = nc.gpsimd if EVAL_NR_ON_GP else nc.vector
        eng.tensor_tensor(t4, d1, y0, OP.mult)
        eng.tensor_tensor(t4, two4, t4, OP.subtract)
        eng.tensor_tensor(t4, y0, t4, OP.mult)
        rat = et.tile([P, B4, NE], f32, tag="rat", name="rat")
        nc.vector.tensor_tensor(rat, m, t4, OP.mult)

        pt = outp.tile([P, B4, NB], f32, tag="pt")
        nc.vector.tensor_tensor(pt, aCY[:, :, 1:NE], aCY[:, :, 0:NB],
                                OP.subtract)
        dr_ = outp.tile([P, B4, NB], f32, tag="dr")
        nc.vector.tensor_tensor(dr_, rat[:, :, 1:NE], rat[:, :, 0:NB],
                                OP.subtract)
        nc.vector.tensor_tensor(pt, pt, dr_, OP.add)
        # scale by 1/Sh first so the clamp threshold is constant 1e-8
        # (rsh broadcast materialized on ACT; stride-0 TT operands are
        # not HW-safe on DVE)
        rshT = outp.tile([P, B4, NB], f32, tag="rshT")
        for bb in range(B4):
            nc.scalar.activation(rshT[:, bb], onesNB, AF.Copy,
                                 bias=0.0, scale=rsh_aps[bb])
        nc.vector.tensor_tensor(pt, pt, rshT, OP.mult)
        nc.vector.tensor_scalar(pt, pt, 1e-8, None, op0=OP.max)
        ot = outp.tile([P, B4, NB], f32, tag="ot")
        nc.scalar.activation(ot, pt, AF.Ln)
        if FANCY_OUT_DMA:
            out_view = out[r0:r0 + B4 * P, :].rearrange(
                "(b p) n -> p b n", b=B4)
            nc.sync.dma_start(out=out_view, in_=ot)
        else:
            for bb in range(B4):
                rb = r0 + bb * P
                nc.sync.dma_start(out=out[rb:rb + P, :], in_=ot[:, bb])

    for sb in range(nsb):
        # ---------------- phase 1: params = softplus(h @ W.T + b) ----------
        params = bs.tile([P, G, ODIM], f32, tag="params")
        usb = bs.tile([P, G, ODIM], f32, tag="usb")
        for g in range(G):
            blk = sb * G + g
            r0 = blk * P
            ht = hpool.tile([P, IN_DIM], f32, tag="ht")
            nc.sync.dma_start(out=ht, in_=h[r0:r0 + P, :])
            psT = psum.tile([P, 2, P], f32, tag="psT")
            for k in range(2):
                nc.tensor.transpose(psT[:, k], ht[:, k * P:(k + 1) * P], ident)
            hT = hpool.tile([P, 2, P], f32, tag="hT")
            nc.scalar.copy(hT, psT)
            pp = psum.tile([P, ODIM], f32, tag="pp")
            nc.tensor.matmul(pp, hT[:, 0], wT[:, 0], start=True, stop=False)
            nc.tensor.matmul(pp, hT[:, 1], wT[:, 1], start=False, stop=False)
            nc.tensor.matmul(pp, ones1, brow, start=False, stop=True)
            expt = usb[:, g]
            nc.scalar.activation(expt, pp, AF.Exp)
            lu = hpool.tile([P, ODIM], f32, tag="lu")
            nc.scalar.activation(lu, expt, AF.Ln)
            dz = hpool.tile([P, ODIM], f32, tag="dz")
            nc.vector.tensor_tensor(dz, pp, lu, OP.subtract)
            nc.vector.tensor_tensor(dz, expt, dz, OP.mult)
            nc.vector.tensor_tensor(expt, expt, dz, OP.add)
            nc.scalar.activation(params[:, g], expt, AF.Ln, bias=1.0, scale=1.0)

        # log1p series fixup for small softplus outputs
        u2sb = bs.tile([P, G, ODIM], f32, tag="u2sb")
        nc.vector.tensor_tensor(u2sb, usb, usb, OP.mult)
        pA = bs.tile([P, G, ODIM], f32, tag="pA")
        nc.vector.tensor_scalar(pA, usb, -0.5, 1.0, op0=OP.mult, op1=OP.add)
        pB = bs.tile([P, G, ODIM], f32, tag="pB")
        nc.vector.tensor_scalar(pB, usb, -0.25, 1.0 / 3, op0=OP.mult, op1=OP.add)
        pC = bs.tile([P, G, ODIM], f32, tag="pC")
        nc.vector.tensor_scalar(pC, usb, -1.0 / 6, 0.2, op0=OP.mult, op1=OP.add)
        u4sb = bs.tile([P, G, ODIM], f32, tag="u4sb")
        nc.vector.tensor_tensor(u4sb, u2sb, u2sb, OP.mult)
        nc.vector.tensor_tensor(pB, pB, u2sb, OP.mult)
        nc.vector.tensor_tensor(pA, pA, pB, OP.add)
        nc.vector.tensor_tensor(pC, pC, u4sb, OP.mult)
        nc.vector.tensor_tensor(pA, pA, pC, OP.add)
        nc.vector.tensor_tensor(pA, pA, usb, OP.mult)
        msk = bs.tile([P, G, ODIM], mybir.dt.uint8, tag="msk")
        nc.vector.tensor_scalar(msk, usb, 0.0625, None, op0=OP.is_lt)
        nc.vector.copy_predicated(params, msk, pA)

        # ---------------- phase 2: bin-space coefficients -------------------
        w_in = bs.tile([P, G, NK], f32, tag="w_in")
        nc.vector.tensor_scalar(w_in, params[:, :, 0:NK], 1e-4, None, op0=OP.add)
        h_in = bs.tile([P, G, NK], f32, tag="h_in")
        nc.vector.tensor_scalar(h_in, params[:, :, NK:2 * NK], 1e-4, None, op0=OP.add)
        dpad = bs.tile([P, G, NK + 2], f32, tag="dpad")
        nc.vector.memset(dpad, 1.0)
        nc.vector.tensor_scalar(
            dpad[:, :, 1:NK + 1], params[:, :, 2 * NK:3 * NK], 1e-4, None, op0=OP.add)

        cx = bs.tile([P, G, NK], f32, tag="cx")
        nc.vector.tensor_tensor_scan(
            cx.rearrange("p g k -> p (g k)"),
            gate.rearrange("p g k -> p (g k)"),
            w_in.rearrange("p g k -> p (g k)"),
            0.0, op0=OP.mult, op1=OP.add)
        cy = bs.tile([P, G, NK], f32, tag="cy")
        nc.vector.tensor_tensor_scan(
            cy.rearrange("p g k -> p (g k)"),
            gate.rearrange("p g k -> p (g k)"),
            h_in.rearrange("p g k -> p (g k)"),
            0.0, op0=OP.mult, op1=OP.add)

        # negated thresholds for ACT mask bias: -cx_j
        ncx = bs.tile([P, G, NK - 1], f32, tag="ncx")
        nc.vector.tensor_scalar(ncx, cx[:, :, 0:NK - 1], -1.0, None, op0=OP.mult)

        rw = bs.tile([P, G, NK], f32, tag="rw")
        nc.vector.reciprocal(rw, w_in)
        rsh = bs.tile([P, G], f32, tag="rsh")
        nc.vector.reciprocal(rsh, cy[:, :, NK - 1])
        lam = bs.tile([P, G], f32, tag="lam")
        nc.vector.tensor_tensor(lam, cx[:, :, NK - 1], rsh, OP.mult)

        dd = bs.tile([P, G, NK], f32, tag="dd")
        nc.vector.tensor_tensor(dd, h_in, rw, OP.mult)
        for g in range(G):
            nc.vector.tensor_scalar(dd[:, g], dd[:, g], lam[:, g:g + 1], None,
                                    op0=OP.mult)

        dl = dpad[:, :, 0:NK]
        dr = dpad[:, :, 1:NK + 1]
        s_t = bs.tile([P, G, NK], f32, tag="s_t")
        nc.vector.tensor_tensor(s_t, dl, dr, OP.add)
        nc.vector.scalar_tensor_tensor(s_t, dd, -2.0, s_t, op0=OP.mult, op1=OP.add)

        apc = bs.tile([P, G, NK], f32, tag="apc")     # A' = h*(dd - dl)
        nc.vector.tensor_tensor(apc, dd, dl, OP.subtract)
        nc.vector.tensor_tensor(apc, apc, h_in, OP.mult)
        bpc = bs.tile([P, G, NK], f32, tag="bpc")     # B' = h*dl*w
        nc.vector.tensor_tensor(bpc, h_in, dl, OP.mult)
        nc.vector.tensor_tensor(bpc, bpc, w_in, OP.mult)

        w2 = bs.tile([P, G, NK], f32, tag="w2")
        nc.vector.tensor_tensor(w2, w_in, w_in, OP.mult)
        c0 = bs.tile([P, G, NK], f32, tag="c0")
        nc.vector.tensor_tensor(c0, dd, w2, OP.mult)
        c1 = bs.tile([P, G, NK], f32, tag="c1")
        nc.vector.tensor_tensor(c1, s_t, w_in, OP.mult)
        c2 = bs.tile([P, G, NK], f32, tag="c2")
        nc.vector.tensor_scalar(c2, s_t, -1.0, None, op0=OP.mult)

        # chain tables: dq_all[p, g, ci, j] (jump at knot j+1), bases[p, g, ci]
        coefs = [None, None, apc, bpc, c0, c1, c2]
        dq_all = bs.tile([P, G, NCH, NK - 1], f32, tag="dq_all")
        nc.vector.tensor_scalar(dq_all[:, :, 0], w_in[:, :, 0:NK - 1], 1.0,
                                None, op0=OP.mult)
        nc.vector.tensor_scalar(dq_all[:, :, 1], h_in[:, :, 0:NK - 1], 1.0,
                                None, op0=OP.mult)
        for ci in range(2, NCH):
            q = coefs[ci]
            nc.vector.tensor_tensor(dq_all[:, :, ci], q[:, :, 1:NK],
                                    q[:, :, 0:NK - 1], OP.subtract)
        bases = bs.tile([P, G, NCH], f32, tag="bases")
        nc.vector.memset(bases[:, :, 0:2], 0.0)
        for ci in range(2, NCH):
            nc.vector.tensor_scalar(bases[:, :, ci:ci + 1],
                                    coefs[ci][:, :, 0:1], 1.0, None,
                                    op0=OP.mult)

        # ---------------- phase 3: chains + batched evaluation --------------
        # Software pipeline: the (DVE-heavy) eval of group k is emitted
        # after the chains of group k+1, so DVE keeps chain work in
        # flight while GPSIMD finishes its blocks of group k.
        for g4 in range(G // B4):
            acc4 = ev.tile([P, B4, NCH, NE], f32, tag="acc4", name="acc4")
            xt4 = ev.tile([P, B4, NE], f32, tag="xt4", name="xt4")
            gblocks = []
            dblocks = []
            for bb in range(B4):
                g = g4 * B4 + bb
                blk = sb * G + g
                sw = cx[:, g, NK - 1:NK]
                nc.scalar.activation(xt4[:, bb], xconst, AF.Copy, bias=0.0,
                                     scale=sw)
                on_gp = (blk * GP_NUM) % GP_DEN < GP_NUM
                (gblocks if on_gp else dblocks).append((bb, g))
            for bb, g in gblocks + dblocks:
                xt = xt4[:, bb]
                acc = acc4[:, bb]
                on_gp = (bb, g) in gblocks

                if not on_gp:
                    # DVE path: custom fused step ops, j-major interleave
                    nc.vector.tensor_scalar(
                        acc[:, 0], xt, cx[:, g, 0:1], w_in[:, g, 0:1],
                        op0=OP.is_gt, op1=OP.mult)
                    nc.vector.tensor_scalar(
                        acc[:, 1], xt, cx[:, g, 0:1], h_in[:, g, 0:1],
                        op0=OP.is_gt, op1=OP.mult)
                    # chains 2..6 init = base value, broadcast on ACT
                    bview = bases[:, g, 2:NCH].rearrange(
                        "p (c e) -> p c e", e=1).broadcast_to(
                        [P, NCH - 2, NE])
                    nc.scalar.activation(acc[:, 2:NCH], bview, AF.Copy)
                    for ci in range(2, NCH):
                        step_madd(acc[:, ci], xt, cx[:, g, 0:1],
                                  dq_all[:, g, ci, 0:1])
                    for j in range(2, NK):
                        cxs = cx[:, g, j - 1:j]
                        for ci in range(NCH):
                            step_madd(acc[:, ci], xt, cxs,
                                      dq_all[:, g, ci, j - 1:j])
                else:
                    # GPSIMD path: exact masks on ACT, chain-batched madd
                    for j in range(1, NK):
                        mask = es.tile([P, NE], f32, tag="mask", name="mask")
                        nc.scalar.activation(mask, xt, AF.Relu,
                                             bias=ncx[:, g, j - 1:j], scale=1.0)
                        nc.scalar.activation(mask, mask, AF.Sign)
                        mask_b = mask.rearrange(
                            "p (c e) -> p c e", c=1).broadcast_to([P, NCH, NE])
                        dq_b = dq_all[:, g, :, j - 1].rearrange(
                            "p (c e) -> p c e", e=1).broadcast_to([P, NCH, NE])
                        if j == 1:
                            nc.gpsimd.tensor_tensor(acc, mask_b, dq_b, OP.mult)
                        else:
                            term = es.tile([P, NCH, NE], f32, tag="term",
                                           name="term")
                            nc.gpsimd.tensor_tensor(term, mask_b, dq_b, OP.mult)
                            nc.gpsimd.tensor_tensor(acc, acc, term, OP.add)
                    bases_b = bases[:, g].rearrange(
                        "p (c e) -> p c e", e=1).broadcast_to([P, NCH, NE])
                    nc.gpsimd.tensor_tensor(acc, acc, bases_b, OP.add)

            # ---- batched rational evaluation over B4 blocks (DVE) ----
            g0 = g4 * B4
            blk0 = sb * G + g0
            rsh_aps = [rsh[:, g0 + bb:g0 + bb + 1] for bb in range(B4)]
            if pending_eval:
                pending_eval.pop(0)()
            pending_eval.append(
                (lambda a4=acc4, x4=xt4, ra=rsh_aps, r0=blk0 * P:
                 emit_eval(a4, x4, ra, r0)))

    while pending_eval:
        pending_eval.pop(0)()


def make_nc(rows, gpsimd_chains=None):
    import concourse.bacc as bacc
    nc = bacc.Bacc("TRN2", target_bir_lowering=False, debug=False,
                   num_devices=N_CORES)
    h_t = nc.dram_tensor("h", [rows, IN_DIM], f32, kind="ExternalInput").ap()
    W_t = nc.dram_tensor("W", [ODIM, IN_DIM], f32, kind="ExternalInput").ap()
    b_t = nc.dram_tensor("b", [ODIM], f32, kind="ExternalInput").ap()
    out_t = nc.dram_tensor("out", [rows, NB], f32, kind="ExternalOutput").ap()
    with tile.TileContext(nc) as tc:
        with ExitStack() as ctx:
            build_rqs(ctx, tc, h_t, W_t, b_t, out_t, rows)
    nc.compile()
    return nc


_cache = {}


def kernel(h, W, b):
    h = np.ascontiguousarray(h, dtype=np.float32)
    W = np.ascontiguousarray(W, dtype=np.float32)
    b = np.ascontiguousarray(b, dtype=np.float32)
    rows = h.shape[0] // N_CORES
    key = ("nc", rows)
    if key not in _cache:
        _cache[key] = make_nc(rows)
    nc = _cache[key]
    from concourse.bass_utils import run_bass_kernel_spmd
    in_maps = [
        {"h": h[i * rows:(i + 1) * rows], "W": W, "b": b}
        for i in range(N_CORES)
    ]
    res = run_bass_kernel_spmd(nc, in_maps, core_ids=list(range(N_CORES)))
    return np.concatenate([r["out"] for r in res.results], axis=0)


if __name__ == "__main__":
    rng = np.random.default_rng(0)
    h = rng.standard_normal((B_FULL, IN_DIM), dtype=np.float32)
    W = (rng.standard_normal((ODIM, IN_DIM), dtype=np.float32) / 16.0)
    b = rng.standard_normal((ODIM,), dtype=np.float32) * 0.01
    out = kernel(h, W, b)
    print(out.shape, out.dtype, out[:2, :4])
